# revision 12
# baseline (speedup 1.0000x reference)
"""Trainium2 Bass kernel: 3x chained zentorch_qlinear (M=8192, K=N=4096).

Strategy (8 NeuronCores, data-parallel over M; no collectives):
  - Each core gets 1024 rows of the input and the full weight matrix.
  - Host marshals inputs into device-friendly layouts (pure layout/dtype
    transforms, all exact):
      * w2  [ki, j, kb, nin] bf16 — the transposed weight W^T tiled so both
        the layer-1/2 stationary tiles and the layer-3 moving tiles are
        contiguous-per-partition DMA reads (int8 values, exact in bf16).
      * xT  [ki, kb, s, mi] f32 — the core's input slice pre-transposed so
        quantized activations land directly in the [k, m] layout matmuls use.
      * swo_rep/bo_rep [128, N] f32 — output scale/bias replicated across
        partitions for the layer-3 row-wise epilogue.
  - Quantize: ACT f32 -> int8 (scale=1/s_in) uses hardware int8 saturation
    for the clip (exact when zp == 128: clip(round(x/s)+zp,0,255)-zp ==
    clip(round(x/s),-128,127)), then DVE int8 -> bf16 copy (exact ints).
  - Layers 1-2 run in yT orientation: psum[n, m] = sum_k WT[k,n] @ aqT[k,m],
    so each layer's output is already in the layout the next layer consumes.
    Per-n quantize scale/bias ride the ACT per-partition APs.
  - Layer 3 flips orientation: stationary = aqT3[k, m-tile], moving =
    WT[k, n-chunk], psum[m, n] accumulates over k. Output needs NO transpose
    and DMAs contiguously to y. Row-wise scale+bias via two DVE passes
    against replicated swo/bo chunks.
  - Zero device-side weight prep, no XBAR transposes, no PE transposes.
"""

import numpy as np

M, K, N = 8192, 4096, 4096
NCORES = 8
ML = M // NCORES   # 1024 rows per core
NBLK = N // 128    # 32
KBLK = K // 128    # 32
MSLAB = ML // 128  # 8
L3CH = 256         # layer-3 output chunk width (2 n-blocks)
NCH3 = N // L3CH   # 16

_COMPILED = {}


def _build(inv_s: float, zp: float):
    import concourse.bacc as bacc
    import concourse.mybir as mybir
    import concourse.tile as tile

    dt = mybir.dt
    AF = mybir.ActivationFunctionType
    Alu = mybir.AluOpType

    i8_path = (zp == 128.0)

    nc = bacc.Bacc("TRN2", target_bir_lowering=False, debug=False, num_devices=NCORES)

    xT = nc.dram_tensor("xt", [128, KBLK, MSLAB, 128], dt.float32, kind="ExternalInput")
    w2 = nc.dram_tensor("w2", [128, NBLK, KBLK, 128], dt.bfloat16, kind="ExternalInput")
    # per-output-channel quantize vectors arranged [p, j]: col j = v[j*128:(j+1)*128]
    swq = nc.dram_tensor("swq", [128, NBLK], dt.float32, kind="ExternalInput")
    bq2 = nc.dram_tensor("bq2", [128, NBLK], dt.float32, kind="ExternalInput")
    swo_rep = nc.dram_tensor("swo_rep", [128, N], dt.float32, kind="ExternalInput")
    bo_rep = nc.dram_tensor("bo_rep", [128, N], dt.float32, kind="ExternalInput")
    y = nc.dram_tensor("y", [ML, N], dt.float32, kind="ExternalOutput")

    JPRE = 3  # layer-1 j-blocks interleaved k-outer during x-prep

    with tile.TileContext(nc) as tc:
        with (
            tc.tile_pool(name="consts", bufs=1) as cpool,
            tc.tile_pool(name="aq", bufs=1) as aqpool,
            tc.tile_pool(name="stat", bufs=JPRE) as statpool,
            tc.tile_pool(name="wsl", bufs=2) as wslpool,
            tc.tile_pool(name="rowc", bufs=2) as rowpool,
            tc.tile_pool(name="mm", bufs=1, space="PSUM") as mmpool,
            tc.tile_pool(name="l3p", bufs=2, space="PSUM") as l3pool,
            tc.tile_pool(name="q", bufs=2) as qpool,
            tc.tile_pool(name="yst", bufs=3) as ystpool,
        ):
            swq_t = cpool.tile([128, NBLK], dt.float32)
            bq2_t = cpool.tile([128, NBLK], dt.float32)
            nc.sync.dma_start(out=swq_t[:], in_=swq[:])
            nc.sync.dma_start(out=bq2_t[:], in_=bq2[:])
            zp_col = None
            if not i8_path:
                zp_col = cpool.tile([128, 1], dt.float32)
                nc.gpsimd.memset(zp_col[:], zp)

            # activations, transposed: [k within blk, k_blk, m_slab, m within slab]
            aqA = aqpool.tile([128, KBLK, MSLAB, 128], dt.bfloat16, name="aqA")
            aqB = aqpool.tile([128, KBLK, MSLAB, 128], dt.bfloat16, name="aqB")

            # ---- X prep: quantize straight in transposed layout (high-priority
            # per-kb DMAs; L1's first JPRE j-blocks interleave k-outer below)
            with tc.tile_pool(name="prep", bufs=3) as ppool:
                for kb in range(KBLK):
                    xs = ppool.tile([128, MSLAB, 128], dt.float32, name="xs", tag="xs")
                    with tc.high_priority():
                        nc.sync.dma_start(out=xs[:], in_=xT[:, kb, :, :])
                    if i8_path:
                        qi = ppool.tile([128, MSLAB, 128], dt.int8, name="qi", tag="qi")
                        nc.scalar.activation(qi[:], xs[:], AF.Identity, scale=inv_s)
                        nc.vector.tensor_copy(aqA[:, kb, :, :], qi[:])
                    else:
                        qu = ppool.tile([128, MSLAB, 128], dt.uint8, name="qu", tag="qu")
                        nc.scalar.activation(
                            qu[:], xs[:], AF.Identity, bias=zp_col[:, 0:1],
                            scale=inv_s,
                        )
                        nc.vector.tensor_scalar(
                            aqA[:, kb, :, :], qu[:], zp, None, Alu.subtract
                        )

            # ---- Layers 1-2 (yT orientation; weight stationary, act moving)
            def quantize_block(OUT, j, ps):
                for h in range(2):
                    if i8_path:
                        qi = qpool.tile([128, 512], dt.int8, name="qh", tag="qh")
                        nc.scalar.activation(
                            qi[:], ps[h][:], AF.Identity,
                            bias=bq2_t[:, j : j + 1], scale=swq_t[:, j : j + 1],
                        )
                        nc.vector.tensor_copy(OUT[:, j, 4 * h : 4 * h + 4, :], qi[:])
                    else:
                        qu = qpool.tile([128, 512], dt.uint8, name="qh", tag="qh")
                        nc.scalar.activation(
                            qu[:], ps[h][:], AF.Identity,
                            bias=bq2_t[:, j : j + 1], scale=swq_t[:, j : j + 1],
                        )
                        nc.vector.tensor_scalar(
                            OUT[:, j, 4 * h : 4 * h + 4, :], qu[:], zp, None,
                            Alu.subtract,
                        )

            def load_stat(j):
                stat = statpool.tile([128, KBLK, 128], dt.bfloat16, name="stat",
                                     tag="stat")
                nc.sync.dma_start(out=stat[:, 0:16, :], in_=w2[:, j, 0:16, :])
                nc.sync.dma_start(out=stat[:, 16:32, :], in_=w2[:, j, 16:32, :])
                return stat

            def new_ps(j):
                return [
                    mmpool.tile([128, 512], dt.float32, name=f"ps{j % JPRE}_{h}",
                                tag=f"ps{j % JPRE}_{h}")
                    for h in range(2)
                ]

            for l in range(2):
                IN = aqA if l == 0 else aqB
                OUT = aqB if l == 0 else aqA
                if l == 0:
                    # Phase A: first JPRE j-blocks k-outer so each arriving
                    # x-prep kb-strip unlocks 2*JPRE matmuls.
                    with tc.high_priority():
                        stats = [load_stat(j) for j in range(JPRE)]
                    pss = [new_ps(j) for j in range(JPRE)]
                    for k in range(KBLK):
                        for j in range(JPRE):
                            for h in range(2):
                                nc.tensor.matmul(
                                    pss[j][h][:],
                                    stats[j][:, k, :],
                                    IN[:, k, 4 * h : 4 * h + 4, :],
                                    start=(k == 0),
                                    stop=(k == KBLK - 1),
                                )
                    for j in range(JPRE):
                        quantize_block(OUT, j, pss[j])
                    jstart = JPRE
                else:
                    jstart = 0
                for j in range(jstart, NBLK):
                    stat = load_stat(j)
                    ps = new_ps(j)
                    for k in range(KBLK):
                        for h in range(2):
                            nc.tensor.matmul(
                                ps[h][:],
                                stat[:, k, :],
                                IN[:, k, 4 * h : 4 * h + 4, :],
                                start=(k == 0),
                                stop=(k == KBLK - 1),
                            )
                    quantize_block(OUT, j, ps)

            # ---- Layer 3 (m-orientation; act stationary, weight moving)
            for c in range(NCH3):
                wsl = wslpool.tile([128, 2, KBLK, 128], dt.bfloat16, name="wsl", tag="wsl")
                swoc = rowpool.tile([128, L3CH], dt.float32, name="swoc", tag="swoc")
                boc = rowpool.tile([128, L3CH], dt.float32, name="boc", tag="boc")
                # Floor-ts so L3 prefetches never steal front DMA bandwidth
                # from x-prep (first consumed at ~930us; 120us floor is safe).
                with tc.tile_wait_until(0.12):
                    for jj in range(2):
                        for hh in range(2):
                            ksl = slice(hh * 16, (hh + 1) * 16)
                            nc.sync.dma_start(
                                out=wsl[:, jj, ksl, :], in_=w2[:, 2 * c + jj, ksl, :]
                            )
                    nc.sync.dma_start(
                        out=swoc[:], in_=swo_rep[:, c * L3CH : (c + 1) * L3CH]
                    )
                    nc.sync.dma_start(
                        out=boc[:], in_=bo_rep[:, c * L3CH : (c + 1) * L3CH]
                    )
                for mt in range(MSLAB):
                    ps3 = l3pool.tile([128, L3CH], dt.float32, name="ps3", tag="ps3")
                    for kb in range(KBLK):
                        nc.tensor.matmul(
                            ps3[:],
                            aqA[:, kb, mt, :],
                            wsl[:, :, kb, :],
                            start=(kb == 0),
                            stop=(kb == KBLK - 1),
                        )
                    yt = ystpool.tile([128, L3CH], dt.float32, name="yt", tag="yt")
                    nc.vector.tensor_tensor(yt[:], ps3[:], swoc[:], Alu.mult)
                    nc.vector.tensor_tensor(yt[:], yt[:], boc[:], Alu.add)
                    nc.sync.dma_start(
                        out=y[mt * 128 : (mt + 1) * 128, c * L3CH : (c + 1) * L3CH],
                        in_=yt[:],
                    )

    nc.compile()
    return nc


def _marshal(input, weights, biases, input_scales, input_zero_points, weight_scales):
    """Host-side layout/dtype marshaling (exact transforms only)."""
    import ml_dtypes

    x = np.asarray(input, dtype=np.float32)
    w = np.asarray(weights, dtype=np.int32)
    b = np.asarray(biases, dtype=np.float32)
    s_in = np.float32(np.asarray(input_scales).reshape(-1)[0])
    zp_in = float(np.asarray(input_zero_points).reshape(-1)[0])
    s_w = np.asarray(weight_scales, dtype=np.float32)

    # w2 [ki, j, kb, nin] bf16 (exact: |w| <= 128)
    w2 = np.ascontiguousarray(
        w.reshape(NBLK, 128, KBLK, 128).transpose(3, 0, 2, 1).astype(np.float32)
    ).astype(ml_dtypes.bfloat16)

    def cols(v):
        return np.ascontiguousarray(v.reshape(NBLK, 128).T.astype(np.float32))

    swq_v = cols(s_w)
    bq2_v = cols(b / s_in) if zp_in == 128.0 else cols(b / s_in + np.float32(zp_in))
    swo_rep = np.ascontiguousarray(
        np.broadcast_to((s_w * s_in)[None, :], (128, N)).astype(np.float32)
    )
    bo_rep = np.ascontiguousarray(np.broadcast_to(b[None, :], (128, N)).astype(np.float32))

    in_maps = []
    for i in range(NCORES):
        xs = x[i * ML : (i + 1) * ML]
        xT = np.ascontiguousarray(
            xs.reshape(MSLAB, 128, KBLK, 128).transpose(3, 2, 0, 1)
        )
        in_maps.append({
            "xt": xT,
            "w2": w2,
            "swq": swq_v,
            "bq2": bq2_v,
            "swo_rep": swo_rep,
            "bo_rep": bo_rep,
        })
    inv_s = float(np.float32(1.0) / s_in)
    return in_maps, inv_s, zp_in


def kernel(input, weights, biases, input_scales, input_zero_points,
           weight_scales, weight_zero_points, output_dtype=None):
    from concourse.bass_utils import run_bass_kernel_spmd

    in_maps, inv_s, zp_in = _marshal(
        input, weights, biases, input_scales, input_zero_points, weight_scales
    )
    key = (inv_s, zp_in)
    if key not in _COMPILED:
        _COMPILED[key] = _build(inv_s, zp_in)
    nc = _COMPILED[key]

    res = run_bass_kernel_spmd(nc, in_maps, core_ids=list(range(NCORES)))
    out = np.concatenate([res.results[i]["y"] for i in range(NCORES)], axis=0)
    return out.astype(np.float32)


if __name__ == "__main__":
    rng = np.random.default_rng(0)
    inp = {
        "input": rng.normal(size=(M, K)).astype(np.float32),
        "weights": rng.integers(-128, 128, (N, K), dtype=np.int32),
        "biases": (rng.normal(size=(N,)) * 0.1).astype(np.float32),
        "input_scales": np.array([0.05], np.float32),
        "input_zero_points": np.array([128], np.int32),
        "weight_scales": rng.uniform(0.001, 0.01, (N,)).astype(np.float32),
        "weight_zero_points": np.zeros((N,), np.int32),
        "output_dtype": 0,
    }
    out = kernel(**inp)
    print(out.shape, out.dtype, np.abs(out).mean())
